# revision 1
# baseline (speedup 1.0000x reference)
"""Ising log-energy kernel for Trainium2 (8 NeuronCores).

Reference computation (B=512 samples, N=4096 spins on a 64x64 grid):
    e[b] = sum_i u[i]*x[b,i] + sum_{i<j} (binary*mask)[i,j]*x[b,i]*x[b,j]

The mask is the nearest-neighbor upper-triangular grid mask: the only
nonzeros of w = binary*mask sit on the +1 and +64 off-diagonals. So

    e[b] = sum_i u[i]*x[b,i] + wr[i]*x[b,i]*x[b,i+1] + wd[i]*x[b,i]*x[b,i+64]

with wr/wd the masked diagonals of `binary`. That's O(B*N) work.

Distribution: tensor-parallel over sites. Core c owns sites
[c*512, c*512+512) for all 512 samples; partial energies are summed on
the host. On-device layout is site-major ([site, batch] = [partition,
free]) so per-site weights ride as matmul lhsT columns and the site sum
is the PE's partition contraction:

  per 128-site chunk k (4 per core), T0/T1/T64 = x rows shifted 0/+1/+64
  (bf16 — exact for +-1 spins):
    DVE : q1  = T0*T1,  q64 = T0*T64          (bf16, exact)
    PE  : acc[2,512] += [w_hi|w_lo].T @ {T0, q1, q64}

  Each fp32 weight vector is split w = hi + lo into two bf16 columns
  (relative error ~2^-17); both columns contract in a single matmul and
  the host adds the two PSUM rows.
"""

import os
from contextlib import ExitStack
import sys

import numpy as np

for _p in ("/opt/trn_rl_repo", "/root/.axon_site/_ro/trn_rl_repo"):
    if os.path.isdir(_p) and _p not in sys.path:
        sys.path.insert(0, _p)

import ml_dtypes

import concourse.bass as bass
import concourse.mybir as mybir
from concourse.bass_utils import run_bass_kernel_spmd


N = 4096          # total spins (64x64 grid)
NG = 64           # grid side (down-neighbor stride)
B = 512           # batch
NCORES = 8
S = N // NCORES   # sites per core = 512
PAD = NG          # extra x rows needed for the +64 shift
NCHUNK = S // 128  # 128-site chunks per core = 4

FP32 = mybir.dt.float32
BF16 = mybir.dt.bfloat16


def _build_bass():
    """Raw Bass (no Tile): the local walrus build only encodes ONE sync
    wait per instruction, so all waits are single cumulative-value waits
    on one of three counting semaphores (dma/dve/pe)."""
    nc = bass.Bass()
    xt = nc.declare_dram_parameter("xt", [S + PAD, B], BF16, isOutput=False)
    wts = nc.declare_dram_parameter("wts", [128, 6 * NCHUNK], BF16, isOutput=False)
    out = nc.declare_dram_parameter("out", [2, B], FP32, isOutput=True)

    with (
        nc.sbuf_tensor("w", [128, 6 * NCHUNK], BF16) as w,
        nc.sbuf_tensor("t0", [128, NCHUNK * B], BF16) as t0,
        nc.sbuf_tensor("t1", [128, NCHUNK * B], BF16) as t1,
        nc.sbuf_tensor("t64", [128, NCHUNK * B], BF16) as t64,
        nc.sbuf_tensor("q1", [128, NCHUNK * B], BF16) as q1,
        nc.sbuf_tensor("q64", [128, NCHUNK * B], BF16) as q64,
        nc.sbuf_tensor("res", [2, B], FP32) as res,
        nc.psum_tensor("acc", [2, B], FP32) as acc,
        nc.semaphore("wsem") as wsem,
        nc.semaphore("osem") as osem,
        nc.semaphore("vsem") as vsem,
        nc.semaphore("psem") as psem,
        nc.semaphore("st0") as st0,
        nc.semaphore("st1") as st1,
        nc.semaphore("st64") as st64,
        nc.Block() as block,
    ):
        # Each DMA gets its own semaphore: concurrent DMAs interleave
        # their 16 per-SDMA-engine sub-increments, so only a semaphore's
        # final total is a race-free wait value. Standalone wait_ge
        # instructions keep everything at walrus's one-sync-wait limit.
        #
        # Each x-shift is ONE fat strided DMA (partition p <- xt rows
        # {shift + p + 128k}): the cost model charges ~650ns of sequencer
        # issue per dma_start, so few fat DMAs beat many small ones. The
        # three loads are split across the two HWDGE queues (SP + ACT) to
        # overlap issue latency.
        def ch(tt, k):
            return tt[:, k * B : (k + 1) * B]

        def load(eng, tile, s, sem, half=None):
            # half=None: all NCHUNK chunks in one DMA; half=0/1: the low/
            # high two chunks, so consumers can start after half the bytes.
            nk, k0 = (NCHUNK, 0) if half is None else (NCHUNK // 2, half * 2)
            eng.dma_start(
                out=tile[:, k0 * B : (k0 + nk) * B].rearrange(
                    "p (k b) -> p k b", k=nk
                ),
                in_=xt[s + k0 * 128 : s + (k0 + nk) * 128].rearrange(
                    "(k p) b -> p k b", p=128
                ),
            ).then_inc(sem, 16)

        # Queue/order choice: w first on the ACT queue (tiny, clears the
        # DMA engines fast), t0 first on the SP queue, then the t1/t64
        # halves interleaved across both queues. The DMA engines serialize
        # at ~350 GB/s in arrival order, so this puts the tensors on the
        # critical path (t1 gates all DVE work) earliest.
        @block.sync
        def _(sync):
            load(sync, t0, 0, st0)
            load(sync, t64, NG, st64, half=0)
            load(sync, t64, NG, st64, half=1)
            sync.wait_ge(vsem, 2 * NCHUNK + 1)
            sync.dma_start(out=out[:], in_=res[:]).then_inc(osem, 16)
            sync.wait_ge(osem, 16)

        @block.scalar
        def _(scalar):
            scalar.dma_start(out=w[:], in_=wts[:]).then_inc(wsem, 16)
            load(scalar, t1, 1, st1, half=0)
            load(scalar, t1, 1, st1, half=1)

        @block.vector
        def _(vector):
            vector.wait_ge(st0, 16)
            vector.wait_ge(st1, 16)
            for k in range(2):
                vector.tensor_mul(ch(q1, k), ch(t0, k), ch(t1, k)).then_inc(vsem, 1)
            vector.wait_ge(st64, 16)
            for k in range(2):
                vector.tensor_mul(ch(q64, k), ch(t0, k), ch(t64, k)).then_inc(
                    vsem, 1
                )
            vector.wait_ge(st1, 32)
            for k in range(2, NCHUNK):
                vector.tensor_mul(ch(q1, k), ch(t0, k), ch(t1, k)).then_inc(vsem, 1)
            vector.wait_ge(st64, 32)
            for k in range(2, NCHUNK):
                vector.tensor_mul(ch(q64, k), ch(t0, k), ch(t64, k)).then_inc(
                    vsem, 1
                )
            vector.wait_ge(psem, 3 * NCHUNK)
            vector.tensor_copy(out=res[:], in_=acc[:]).then_inc(vsem, 1)

        @block.tensor
        def _(tensor):
            tensor.wait_ge(wsem, 16)
            tensor.wait_ge(st0, 16)
            # u-term matmuls first (need only w+t0), then the product
            # matmuls in the exact order DVE emits them (vsem counts).
            n_mm = 0
            for k in range(NCHUNK):
                tensor.matmul(
                    acc[:],
                    w[:, 6 * k : 6 * k + 2],
                    ch(t0, k),
                    start=(n_mm == 0),
                    stop=False,
                ).then_inc(psem, 1)
                n_mm += 1
            prod_order = [("q1", 0), ("q1", 1), ("q64", 0), ("q64", 1),
                          ("q1", 2), ("q1", 3), ("q64", 2), ("q64", 3)]
            qt = {"q1": (q1, 2), "q64": (q64, 4)}
            for i, (name, k) in enumerate(prod_order):
                tile, woff = qt[name]
                tensor.wait_ge(vsem, i + 1)
                tensor.matmul(
                    acc[:],
                    w[:, 6 * k + woff : 6 * k + woff + 2],
                    ch(tile, k),
                    start=False,
                    stop=(i == len(prod_order) - 1),
                ).then_inc(psem, 1)
                n_mm += 1

    return nc


_NC_CACHE = None


def _get_nc():
    global _NC_CACHE
    if _NC_CACHE is None:
        _NC_CACHE = _build_bass()
    return _NC_CACHE


def _split_bf16(v):
    """fp32 vector -> (hi, lo) bf16 pair with hi+lo ~= v to ~2^-17 rel."""
    hi = v.astype(ml_dtypes.bfloat16)
    lo = (v - hi.astype(np.float32)).astype(ml_dtypes.bfloat16)
    return hi, lo


def _prep_inputs(x, unary, binary, mask):
    """Host-side shard prep: masked diagonals + padded transposed spins."""
    wr = np.zeros(N, np.float32)
    wd = np.zeros(N, np.float32)
    wr[: N - 1] = np.diagonal(binary, 1) * np.diagonal(mask, 1)
    wd[: N - NG] = np.diagonal(binary, NG) * np.diagonal(mask, NG)
    u = np.asarray(unary, np.float32)

    xt = np.zeros((N + PAD, B), ml_dtypes.bfloat16)
    xt[:N] = np.asarray(x, np.float32).T.astype(ml_dtypes.bfloat16)

    in_maps = []
    for c in range(NCORES):
        base = c * S
        w = np.empty((128, 6 * NCHUNK), ml_dtypes.bfloat16)
        for k in range(NCHUNK):
            rows = slice(base + k * 128, base + k * 128 + 128)
            for t, vec in enumerate((u, wr, wd)):
                hi, lo = _split_bf16(vec[rows])
                w[:, 6 * k + 2 * t] = hi
                w[:, 6 * k + 2 * t + 1] = lo
        in_maps.append(
            {"xt": np.ascontiguousarray(xt[base : base + S + PAD]), "wts": w}
        )
    return in_maps


def kernel(x, unary, binary, mask):
    nc = _get_nc()
    in_maps = _prep_inputs(x, unary, binary, mask)
    res = run_bass_kernel_spmd(nc, in_maps, list(range(NCORES))).results
    parts = np.stack([r["out"] for r in res])  # [8, 2, B]
    return parts.sum(axis=(0, 1), dtype=np.float64).astype(np.float32)



# revision 14
# speedup vs baseline: 1.2671x; 1.2671x over previous
"""Ising log-energy kernel for Trainium2 (8 NeuronCores).

Reference computation (B=512 samples, N=4096 spins on a 64x64 grid):
    e[b] = sum_i u[i]*x[b,i] + sum_{i<j} (binary*mask)[i,j]*x[b,i]*x[b,j]

The mask is the nearest-neighbor upper-triangular grid mask: the only
nonzeros of w = binary*mask sit on the +1 and +64 off-diagonals. So

    e[b] = sum_i u[i]*x[b,i] + wr[i]*x[b,i]*x[b,i+1] + wd[i]*x[b,i]*x[b,i+64]

with wr/wd the masked diagonals of `binary`. That's O(B*N) work.

Distribution: tensor-parallel over sites. Core c owns sites
[c*512, c*512+512) for all 512 samples; partial energies are summed on
the host. On-device layout is site-major ([site, batch] = [partition,
free]).

Everything on device is fp8e4m3, which represents +-1 spins EXACTLY:
  - DMA ships half the bytes of bf16 (3 x 256KB per core).
  - Spin products use the sign-bit XOR identity: for a,b in {+-1},
    fp8(a*b) = byte(a) XOR byte(b) XOR 0x38. The host pre-XORs the
    shifted operands with 0x38 (making them "sign-only" bytes), so the
    device computes each product tensor with a single DVE bitwise XOR.
    The XOR is dtype-agnostic, so it runs on uint16-bitcast APs where
    the DVE gets its 2x 2-byte throughput (fp8 tensor_mul would be 1x).
  - PE runs fp8 DoubleRow matmuls (0.5 cycles/row): each of the 6
    matmuls contracts TWO 128-site chunks at once. Weights ride as
    hi+lo fp8 pairs (residual ~2^-8, energy rel err ~1e-3).
  - The [2, 512] fp32 accumulator is DMA'd to DRAM straight out of
    PSUM; the NEFF epilogue's dma_reset drains that in-flight DMA, so
    no engine sits on the completion semaphore.
"""

import os
from contextlib import ExitStack
import sys

import numpy as np

for _p in ("/opt/trn_rl_repo", "/root/.axon_site/_ro/trn_rl_repo"):
    if os.path.isdir(_p) and _p not in sys.path:
        sys.path.insert(0, _p)

import ml_dtypes

import concourse.bass as bass
import concourse.mybir as mybir
from concourse.bass_utils import run_bass_kernel_spmd


N = 4096          # total spins (64x64 grid)
NG = 64           # grid side (down-neighbor stride)
B = 512           # batch
NCORES = 8
S = N // NCORES   # sites per core = 512
NCHUNK = S // 128  # 128-site chunks per core = 4

FP32 = mybir.dt.float32
FP8 = mybir.dt.float8e4
U16 = mybir.dt.uint16
F8NP = ml_dtypes.float8_e4m3

DOUBLE_ROW = True       # fp8 DoubleRow matmuls (2 chunks / instruction)
WAIT_OUT_DMA = False    # rely on NEFF epilogue dma_reset to drain out DMA


def _build_bass():
    """Raw Bass (no Tile): the local walrus build only encodes ONE sync
    wait per instruction, so all waits are single cumulative-value waits
    on counting semaphores."""
    nc = bass.Bass()
    # xin rows: [0:512) = x slice as fp8; [512:1024) = x slice shifted by
    # +1 site, sign-bit-only bytes; [1024:1536) = +64 shift, sign-only.
    xin = nc.declare_dram_parameter("xin", [3 * S, B], FP8, isOutput=False)
    # wts col layout: slot (j, t) at (j*3+t)*32; within a slot the two
    # ktile blocks sit 16 cols apart (DoubleRow wants stride%16==0), each
    # holding the (hi, lo) fp8 split pair.
    wts = nc.declare_dram_parameter("wts", [128, 192], FP8, isOutput=False)
    out = nc.declare_dram_parameter("out", [2, B], FP32, isOutput=True)

    with (
        nc.sbuf_tensor("xb", [128, 3 * NCHUNK * B], FP8) as xb,
        nc.sbuf_tensor("q", [128, 2 * NCHUNK * B], FP8) as q,
        nc.sbuf_tensor("w", [128, 192], FP8) as w,
        nc.sbuf_tensor("res", [2, B], FP32) as res,
        nc.psum_tensor("acc", [2, B], FP32) as acc,
        nc.semaphore("wsem") as wsem,
        nc.semaphore("osem") as osem,
        nc.semaphore("vsem") as vsem,
        nc.semaphore("psem") as psem,
        nc.semaphore("s0") as s0,
        nc.semaphore("s1") as s1,
        nc.semaphore("s64a") as s64a,
        nc.semaphore("s64b") as s64b,
        nc.Block() as block,
    ):
        # One fat strided DMA per x region (row r -> partition r%128,
        # chunk r//128). Rows r0..r1 of xin land at xb columns 4*r0..4*r1.
        def load(eng, r0, r1, sem):
            nk = (r1 - r0) // 128
            eng.dma_start(
                out=xb[:, 4 * r0 : 4 * r1].rearrange("p (k b) -> p k b", k=nk),
                in_=xin[r0:r1].rearrange("(k p) b -> p k b", p=128),
            ).then_inc(sem, 16)

        # Queue balance: sync carries t0 + t64-half0 (384KB), scalar
        # carries w + t1 + t64-half1 (387KB).
        @block.sync
        def _(sync):
            load(sync, 0, S, s0)                     # t0 (plain fp8)
            load(sync, 2 * S, 2 * S + 256, s64a)     # t64 sign-only, chunks 0-1
            sync.wait_ge(vsem, 5)
            sync.dma_start(out=out[:], in_=res[:]).then_inc(osem, 16)
            if WAIT_OUT_DMA:
                sync.wait_ge(osem, 16)

        @block.scalar
        def _(scalar):
            scalar.dma_start(out=w[:], in_=wts[:]).then_inc(wsem, 16)
            load(scalar, S, 2 * S, s1)               # t1 sign-only
            load(scalar, 2 * S + 256, 3 * S, s64b)   # t64 sign-only, chunks 2-3

        # fp8 product of +-1 spins == XOR of (plain, sign-only) bytes.
        # Run it on uint16-bitcast views for the DVE 2-byte fast path.
        def xor(vector, qcol, acol, bcol):
            vector.tensor_tensor(
                q[:, qcol : qcol + 1024].bitcast(U16),
                xb[:, acol : acol + 1024].bitcast(U16),
                xb[:, bcol : bcol + 1024].bitcast(U16),
                mybir.AluOpType.bitwise_xor,
            ).then_inc(vsem, 1)

        @block.vector
        def _(vector):
            vector.wait_ge(s0, 16)
            vector.wait_ge(s1, 16)
            xor(vector, 0, 0, 2048)          # q1 chunks 0-1
            xor(vector, 1024, 1024, 3072)    # q1 chunks 2-3
            vector.wait_ge(s64a, 16)
            xor(vector, 2048, 0, 4096)       # q64 chunks 0-1
            vector.wait_ge(s64b, 16)
            xor(vector, 3072, 1024, 5120)    # q64 chunks 2-3
            vector.wait_ge(psem, 6 if DOUBLE_ROW else 12)
            vector.tensor_copy(out=res[:], in_=acc[:]).then_inc(vsem, 1)

        # Six DoubleRow matmuls: (term, chunk-pair) with term-major PSUM
        # accumulation. lhsT [128, ktile=2, m=2(hi,lo)], rhs [128, 2, 512].
        # Every matmul bumps psem; sync waits for the full count.
        def mm(tensor, slot, tile, col, start, stop):
            wcol = slot * 32
            if DOUBLE_ROW:
                lhsT = w[:, wcol : wcol + 32].rearrange("p (k g) -> p k g", k=2)
                lhsT = lhsT[:, :, 0:2]  # [128, ktile(stride 16), m=2]
                rhs = tile[:, col : col + 1024].rearrange("p (k b) -> p k b", k=2)
                tensor.matmul(
                    acc[:],
                    lhsT,
                    rhs,
                    start=start,
                    stop=stop,
                    perf_mode=mybir.MatmulPerfMode.DoubleRow,
                ).then_inc(psem, 1)
            else:
                for k in range(2):
                    lhsT = w[:, wcol + 16 * k : wcol + 16 * k + 2]
                    rhs = tile[:, col + 512 * k : col + 512 * (k + 1)]
                    tensor.matmul(
                        acc[:], lhsT, rhs, start=start and k == 0, stop=stop and k == 1
                    ).then_inc(psem, 1)

        @block.tensor
        def _(tensor):
            tensor.wait_ge(wsem, 16)
            tensor.wait_ge(s0, 16)
            mm(tensor, 0, xb, 0, True, False)       # u, chunks 0-1
            mm(tensor, 3, xb, 1024, False, False)   # u, chunks 2-3
            tensor.wait_ge(vsem, 1)
            mm(tensor, 1, q, 0, False, False)       # wr, chunks 0-1
            tensor.wait_ge(vsem, 2)
            mm(tensor, 4, q, 1024, False, False)    # wr, chunks 2-3
            tensor.wait_ge(vsem, 3)
            mm(tensor, 2, q, 2048, False, False)    # wd, chunks 0-1
            tensor.wait_ge(vsem, 4)
            mm(tensor, 5, q, 3072, False, True)     # wd, chunks 2-3

    return nc


_NC_CACHE = None


def _get_nc():
    global _NC_CACHE
    if _NC_CACHE is None:
        _NC_CACHE = _build_bass()
    return _NC_CACHE


def _split_fp8(v):
    """fp32 vector -> (hi, lo) fp8e4m3 pair with hi+lo ~= v (~2^-8 rel)."""
    hi = v.astype(F8NP)
    lo = (v - hi.astype(np.float32)).astype(F8NP)
    return hi, lo


def _prep_inputs(x, unary, binary, mask):
    """Host-side shard prep: masked diagonals + fp8 byte-packed spins."""
    wr = np.zeros(N, np.float32)
    wd = np.zeros(N, np.float32)
    wr[: N - 1] = np.diagonal(binary, 1) * np.diagonal(mask, 1)
    wd[: N - NG] = np.diagonal(binary, NG) * np.diagonal(mask, NG)
    u = np.asarray(unary, np.float32)

    # Site-major sign bits, padded so the +64 shift stays in bounds.
    pos = np.zeros((N + NG, B), dtype=bool)
    pos[:N] = np.asarray(x).T > 0
    t0 = np.where(pos, np.uint8(0x38), np.uint8(0xB8))   # fp8(+-1)
    sgn = np.where(pos, np.uint8(0x00), np.uint8(0x80))  # sign-only
    sgn[N:] = 0  # pad rows: +0.0 (their weights are 0)

    in_maps = []
    vecs = (u, wr, wd)
    for c in range(NCORES):
        base = c * S
        xin = np.empty((3 * S, B), np.uint8)
        xin[0:S] = t0[base : base + S]
        xin[S : 2 * S] = sgn[base + 1 : base + S + 1]
        xin[2 * S : 3 * S] = sgn[base + NG : base + S + NG]

        w = np.zeros((128, 192), F8NP)
        for j in range(2):           # chunk pair
            for t in range(3):       # term: u, wr, wd
                for k in range(2):   # ktile within pair
                    rows = slice(base + (2 * j + k) * 128, base + (2 * j + k + 1) * 128)
                    hi, lo = _split_fp8(vecs[t][rows])
                    col = (j * 3 + t) * 32 + k * 16
                    w[:, col] = hi
                    w[:, col + 1] = lo
        in_maps.append({"xin": xin.view(F8NP), "wts": w})
    return in_maps


def kernel(x, unary, binary, mask):
    nc = _get_nc()
    in_maps = _prep_inputs(x, unary, binary, mask)
    res = run_bass_kernel_spmd(nc, in_maps, list(range(NCORES))).results
    parts = np.stack([r["out"] for r in res])  # [8, 2, B]
    return parts.sum(axis=(0, 1), dtype=np.float64).astype(np.float32)
